# revision 6
# baseline (speedup 1.0000x reference)
"""Trainium2 Bass kernel for nn_Decoder2 (GRU decoder, Keras reset_after GRUCell).

Reference computation (per batch row b, scanned over t = 0..T-1):
    x_t   = [o_{t-1}, feat_t]                  # [1+F]
    mx    = x_t @ K + ib                       # [3H]
    mh    = h_{t-1} @ Wr + rb                  # [3H]
    z     = sigmoid(mx[:H]   + mh[:H])
    r     = sigmoid(mx[H:2H] + mh[H:2H])
    cand  = tanh(mx[2H:] + r * mh[2H:])
    h_t   = z * h_{t-1} + (1-z) * cand
    o_t   = h_t @ dw + db                      # scalar output per row

Shapes: B=8192, T=96, F=64, H=256.

Strategy: pure data parallel over batch (1024 rows/core on 8 cores), no
collectives.  On-chip layout is feature-major: [feature -> partitions,
batch -> free dim], so TensorE consumes the tiny weights as stationary
operands and the bias rides along as a ones-row in x.

Perf structure:
  * o-feedback for the z/r gates is folded host-side into the recurrent
    weights (Wr' = Wr + dw@k0^T on the z/r columns), shortening the
    per-step latency chain.  Only the xh pass reads the o row of x.
  * the recurrent gate matmuls run in fp8e4m3 with the DoubleRow perf
    mode: one K=256 pass per 128-col chunk at 0.5 cycles/row (4x fewer
    PE cycles than bf16).  Scales: h*16, wr*32 -> PSUM holds 512x the
    true preactivation; the x-side z/r weights are pre-scaled by 512 to
    match and the sigmoid undoes it via the ACT scale argument.  The
    output path (o) stays bf16.
  * x tile is [66, n]: feat rows 0..63, o row 64, ones row 65.  The
    ones row ships from the host inside featT (no per-step memsets).
  * no separate "xh += r*hh" DVE add: DVE writes rh = r*(hh+rb)
    directly into the xh PSUM bank, and the xh matmul (deferred to the
    next pipeline block so the PE never waits on DVE) accumulates the
    x-side on top with start=False.
  * z and r PSUM share one 4-bank tile; sigmoid(r) is issued first
    (it is on the recurrence critical path), sigmoid(z) after.
  * elementwise work spread across DVE (rh, blends), Pool (fp8 cast of
    h, o->x-row cast), ACT (sigmoids/tanh/o copy).
"""

import os
import sys

for _p in ("/root/.axon_site/_ro/trn_rl_repo", "/opt/trn_rl_repo"):
    if os.path.isdir(_p) and _p not in sys.path:
        sys.path.insert(0, _p)

from contextlib import ExitStack  # noqa: E402

import numpy as np  # noqa: E402

import concourse.bacc as bacc  # noqa: E402
import concourse.tile as tile  # noqa: E402
from concourse import mybir  # noqa: E402
from concourse import bass_utils  # noqa: E402

Alu = mybir.AluOpType
Act = mybir.ActivationFunctionType
DR = mybir.MatmulPerfMode.DoubleRow

B, T, F, H = 8192, 96, 64, 256
G3 = 3 * H              # 768 gate width
NCORES = 8
BL = B // NCORES        # 1024 batch rows per core
# x tile partition layout:
#   rows 0..63  = feat_t^T
#   row  64     = o_{t-1} (prev dense output, raw = o - db)
#   row  65     = ones (bias row, shipped from host inside featT)
XROWS = 66
OROW = 64
ONESROW = 65
SH = 16.0               # fp8 scale on h
SW = 32.0               # fp8 scale on the fp8 recurrent weights
SC = SH * SW            # combined PSUM scale on the z/r preactivations


def build_nc(
    t_steps: int = T,
    bl: int = BL,
    nt: int = 2,
    fp8: str = "zrhh",        # "zr" | "zrhh"  (recurrent matmul precision)
    h8_pool: bool = True,     # fp8 cast of h on gpsimd (else DVE)
    o_single: bool = False,   # single bf16 o copy (outT in bf16)
):
    """Build (and compile) the per-core Bass program.

    With fp8="zrhh" the recurrent bias rb must be zero (asserted in
    make_in_maps): the hh DoubleRow matmul has nowhere to add it.
    """
    n = bl // nt                     # batch-tile free size (<=512: one PSUM bank)
    assert n <= 512
    bf = mybir.dt.bfloat16
    f8 = mybir.dt.float8e4
    f32 = mybir.dt.float32
    hh_fp8 = fp8 == "zrhh"
    nch = H // 128                   # 2 chunks of 128 features for h
    assert nch == 2

    nc = bacc.Bacc("TRN2", target_bir_lowering=False, debug=False)

    featT = nc.dram_tensor("featT", [t_steps, XROWS, bl], bf, kind="ExternalInput").ap()
    h0T = nc.dram_tensor("h0T", [128, nch, bl], bf, kind="ExternalInput").ap()
    h0T8 = nc.dram_tensor("h0T8", [128, nch, bl], f8, kind="ExternalInput").ap()
    # kxw: [66, G3] x-side stationary.  z/r columns pre-scaled by SC.
    # row OROW: zeros for z/r columns (o-feedback folded into wrzr there),
    # kernel[0] for the h columns.  row ONESROW carries the biases.
    kxw = nc.dram_tensor("kxw", [XROWS, G3], bf, kind="ExternalInput").ap()
    # z/r recurrent weights (folded o-feedback), fp8 *SW
    wrzr = nc.dram_tensor("wrzr", [128, nch, 2 * H], f8, kind="ExternalInput").ap()
    # unfolded variant for t=0 (o-feedback comes via kxo from the x row)
    wrzr0 = nc.dram_tensor("wrzr0", [128, nch, 2 * H], f8, kind="ExternalInput").ap()
    # h-column recurrent weights
    wrh = nc.dram_tensor("wrh", [128, nch, H], f8 if hh_fp8 else bf,
                         kind="ExternalInput").ap()
    # k0 z/r columns (*SC) for the t=0 K=1 o-feedback matmul
    kxo = nc.dram_tensor("kxo", [1, 2 * H], bf, kind="ExternalInput").ap()
    dww = nc.dram_tensor("dww", [128, nch], bf, kind="ExternalInput").ap()
    rbh = nc.dram_tensor("rbh", [128, nch], f32, kind="ExternalInput").ap()
    out_dt = bf if o_single else f32
    outT = nc.dram_tensor("outT", [t_steps, bl], out_dt, kind="ExternalOutput").ap()

    with tile.TileContext(nc) as tc, ExitStack() as ctx:
        const = ctx.enter_context(tc.tile_pool(name="const", bufs=1))
        hpool = ctx.enter_context(tc.tile_pool(name="h", bufs=3))
        h8pool = ctx.enter_context(tc.tile_pool(name="h8", bufs=3))
        xpool = ctx.enter_context(tc.tile_pool(name="x", bufs=6))
        zrpool = ctx.enter_context(tc.tile_pool(name="zr", bufs=3))
        apool = ctx.enter_context(tc.tile_pool(name="a", bufs=3))
        bpool = ctx.enter_context(tc.tile_pool(name="b", bufs=3))
        cpool = ctx.enter_context(tc.tile_pool(name="cand", bufs=3))
        opool = ctx.enter_context(tc.tile_pool(name="osb", bufs=4))
        # PSUM: 4 (zr) + 2 (hh, shared with o) + 2 (xq) = 8 banks
        pzr = ctx.enter_context(tc.tile_pool(name="pzr", bufs=1, space="PSUM"))
        phh = ctx.enter_context(tc.tile_pool(name="phh", bufs=1, space="PSUM"))
        pxq = ctx.enter_context(tc.tile_pool(name="pxq", bufs=1, space="PSUM"))

        # --- constants ---
        kx_sb = const.tile([XROWS, G3], bf)
        nc.sync.dma_start(out=kx_sb, in_=kxw)
        wrzr_sb = const.tile([128, nch, 2 * H], f8)
        nc.sync.dma_start(out=wrzr_sb, in_=wrzr)
        wrzr0_sb = const.tile([128, nch, 2 * H], f8)
        nc.sync.dma_start(out=wrzr0_sb, in_=wrzr0)
        wrh_sb = const.tile([128, nch, H], f8 if hh_fp8 else bf)
        nc.sync.dma_start(out=wrh_sb, in_=wrh)
        # k0 z/r row staged at partition OROW so the t=0 K=1 matmul's
        # stationary and moving operands share a row group
        kxo_sb = const.tile([XROWS, 2 * H], bf)
        nc.sync.dma_start(out=kxo_sb[OROW:OROW + 1, :], in_=kxo)
        dw_sb = const.tile([128, nch], bf)
        nc.sync.dma_start(out=dw_sb, in_=dww)
        rb_sb = const.tile([128, nch], f32)
        nc.sync.dma_start(out=rb_sb, in_=rbh)

        # --- initial state ---
        h_prev0 = hpool.tile([128, nch, bl], bf)
        nc.sync.dma_start(out=h_prev0, in_=h0T)
        h8_prev0 = h8pool.tile([128, nch, bl], f8)
        nc.sync.dma_start(out=h8_prev0, in_=h0T8)
        xs = {}
        for j in range(nt):
            xj = xpool.tile([XROWS, n], bf, tag="x")
            nc.sync.dma_start(out=xj, in_=featT[0, :, j * n:(j + 1) * n])
            xs[(0, j)] = xj

        h_hist = {-1: h_prev0}
        h8_hist = {-1: h8_prev0}
        zr_hist = {}
        xq_hist = {}

        def emit_tail(t, j, skip_h8=False):
            """Deferred back half of block (t, j): x-side accumulation onto
            rh in PSUM, tanh, blend, fp8 cast.  Emitted one block later so
            the PE's xh matmul never waits on DVE within its own block."""
            bs = slice(j * n, (j + 1) * n)
            x = xs[(t, j)]
            xq = xq_hist.pop((t, j))
            zr_sb = zr_hist.pop((t, j))
            z_sb = zr_sb[:, 0:nch, :]
            h_prev = h_hist[t - 1]
            h_new = h_hist[t]
            h8_new = h8_hist[t]
            for c in range(nch):
                m = 2 * H + c * 128
                nc.tensor.matmul(xq[:, c, :], kx_sb[:, m:m + 128], x,
                                 start=False, stop=True, skip_group_check=True)
            cand = cpool.tile([128, nch, n], bf, tag="cand")
            nc.scalar.activation(cand, xq, Act.Tanh)
            # h_new = A - (z-1)*cand  with A = z*h (off the tanh chain)
            a_sb = apool.tile([128, nch, n], bf, tag="a")
            nc.vector.tensor_tensor(a_sb, z_sb, h_prev[:, :, bs], Alu.mult)
            b_sb = bpool.tile([128, nch, n], bf, tag="b")
            nc.vector.scalar_tensor_tensor(
                b_sb, z_sb, -1.0, cand, Alu.add, Alu.mult)
            nc.vector.tensor_tensor(h_new[:, :, bs], a_sb, b_sb, Alu.subtract)
            if not skip_h8:
                if h8_pool:
                    nc.gpsimd.tensor_scalar_mul(h8_new[:, :, bs],
                                                h_new[:, :, bs], SH)
                else:
                    nc.vector.tensor_scalar_mul(h8_new[:, :, bs],
                                                h_new[:, :, bs], SH)

        def emit_o_mm(j, h_t):
            """o(t,j) = h(t,j) @ dw into a psum slot shared with hh."""
            bs = slice(j * n, (j + 1) * n)
            op = phh.tile([1, n], f32, tag="phh")
            nc.tensor.matmul(op[0:1, :], dw_sb[:, 0:1], h_t[:, 0, bs],
                             start=True, stop=False)
            nc.tensor.matmul(op[0:1, :], dw_sb[:, 1:2], h_t[:, 1, bs],
                             start=False, stop=True)
            return op

        def emit_o_out(t, j, op, x_next):
            """Copy o(t,j) out of PSUM: to outT and (if x_next) into the o
            row of x(t+1,j).  The PSUM->SBUF copy alternates ACT/DVE per j
            to balance engine load."""
            bs = slice(j * n, (j + 1) * n)
            if o_single:
                dst = x_next[OROW:OROW + 1, :] if x_next is not None else \
                    opool.tile([1, n], bf, tag="osb")
                if j == 0:
                    nc.scalar.activation(dst, op, Act.Copy)
                else:
                    nc.vector.tensor_copy(out=dst, in_=op)
                nc.sync.dma_start(out=outT[t:t + 1, bs], in_=dst)
            else:
                o_sb = opool.tile([1, n], f32, tag="osb")
                if j == 0:
                    nc.scalar.activation(o_sb, op, Act.Copy)
                else:
                    nc.vector.tensor_copy(out=o_sb, in_=op)
                nc.sync.dma_start(out=outT[t:t + 1, bs], in_=o_sb)
                if x_next is not None:
                    nc.gpsimd.tensor_copy(out=x_next[OROW:OROW + 1, :],
                                          in_=o_sb)

        prev = None
        for t in range(t_steps):
            h_hist[t] = hpool.tile([128, nch, bl], bf, tag="h")
            h8_hist[t] = h8pool.tile([128, nch, bl], f8, tag="h8")
            h_prev = h_hist[t - 1]
            h8_prev = h8_hist[t - 1]
            for j in range(nt):
                bs = slice(j * n, (j + 1) * n)
                x = xs[(t, j)]

                # 1. z/r gate matmuls: fp8 DoubleRow recurrent pass + bf16
                # x pass, in a shared 4-bank psum tile (z chunks 0..1,
                # r chunks 2..3).
                zrp = pzr.tile([128, 2 * nch, n], f32, tag="pzr")
                wsrc = wrzr0_sb if t == 0 else wrzr_sb
                for g in range(2 * nch):        # z0 z1 r0 r1
                    m = g * 128
                    nc.tensor.matmul(zrp[:, g, :], wsrc[:, :, m:m + 128],
                                     h8_prev[:, :, bs],
                                     start=True, stop=False, perf_mode=DR)
                    nc.tensor.matmul(zrp[:, g, :], kx_sb[:, m:m + 128], x,
                                     start=False, stop=t != 0)
                    if t == 0:
                        # o-feedback at t=0 is the external init input
                        # (x row OROW); K=1 matmul on PE row-group 64
                        nc.tensor.matmul(
                            zrp[:, g, :], kxo_sb[OROW:OROW + 1, m:m + 128],
                            x[OROW:OROW + 1, :],
                            start=False, stop=True, tile_position=(64, 0))

                # 2-5. deferred tail of the previous block
                if prev is not None:
                    emit_tail(*prev)

                # 6. deferred output work from step t-1 (h ready long ago;
                # must precede the hh matmuls below: shared psum slot)
                if t > 0:
                    op = emit_o_mm(j, h_prev)
                    emit_o_out(t - 1, j, op, x)

                # 7. prefetch next step's x tile (feat + o placeholder + ones)
                if t < t_steps - 1:
                    xj = xpool.tile([XROWS, n], bf, tag="x")
                    nc.sync.dma_start(
                        out=xj, in_=featT[t + 1, :, j * n:(j + 1) * n])
                    xs[(t + 1, j)] = xj

                # 8. hh matmuls
                hhp = phh.tile([128, nch, n], f32, tag="phh")
                for c in range(nch):
                    if hh_fp8:
                        nc.tensor.matmul(
                            hhp[:, c, :], wrh_sb[:, :, c * 128:c * 128 + 128],
                            h8_prev[:, :, bs], start=True, stop=True,
                            perf_mode=DR)
                    else:
                        nc.tensor.matmul(
                            hhp[:, c, :], wrh_sb[:, 0, c * 128:c * 128 + 128],
                            h_prev[:, 0, bs], start=True, stop=False)
                        nc.tensor.matmul(
                            hhp[:, c, :], wrh_sb[:, 1, c * 128:c * 128 + 128],
                            h_prev[:, 1, bs], start=False, stop=True)

                # 9. sigmoids: r first (critical path), z after
                zr_sb = zrpool.tile([128, 2 * nch, n], bf, tag="zr")
                nc.scalar.activation(zr_sb[:, nch:2 * nch, :],
                                     zrp[:, nch:2 * nch, :], Act.Sigmoid,
                                     scale=1.0 / SC)
                nc.scalar.activation(zr_sb[:, 0:nch, :], zrp[:, 0:nch, :],
                                     Act.Sigmoid, scale=1.0 / SC)
                r_sb = zr_sb[:, nch:2 * nch, :]

                # 10. rh = r * (hh + rb) written straight into the xq psum
                # bank; next block's xh matmul accumulates on top.
                xq = pxq.tile([128, nch, n], f32, tag="pxq")
                if hh_fp8:
                    nc.vector.scalar_tensor_tensor(
                        xq, hhp, 1.0 / SC, r_sb, Alu.mult, Alu.mult)
                else:
                    for c in range(nch):
                        nc.vector.scalar_tensor_tensor(
                            xq[:, c, :], hhp[:, c, :], rb_sb[:, c:c + 1],
                            r_sb[:, c, :], Alu.add, Alu.mult)
                xq_hist[(t, j)] = xq
                zr_hist[(t, j)] = zr_sb
                prev = (t, j)

            h_hist.pop(t - 2, None)
            h8_hist.pop(t - 2, None)
            xs.pop((t - 1, 0), None)
            xs.pop((t - 1, 1), None)

        # flush: tail of the last block, then the last step's outputs
        emit_tail(t_steps - 1, nt - 1, skip_h8=True)
        for j in range(nt):
            op = emit_o_mm(j, h_hist[t_steps - 1])
            emit_o_out(t_steps - 1, j, op, None)

    nc.compile()
    return nc


_NC_CACHE: dict = {}


def _get_nc(t_steps=T, bl=BL, nt=2, compute_dt="bfloat16", fp8="zrhh",
            h8_pool=True, o_single=False):
    key = (t_steps, bl, nt, fp8, h8_pool, o_single)
    if key not in _NC_CACHE:
        _NC_CACHE[key] = build_nc(t_steps, bl, nt, fp8, h8_pool, o_single)
    return _NC_CACHE[key]


def make_in_maps(
    decoder_feature,
    init_state,
    decoder_init_input,
    kernel,
    recurrent_kernel,
    input_bias,
    recurrent_bias,
    dense_w,
    dense_b,
    t_steps=T,
    bl=BL,
    ncores=NCORES,
    fp8="zrhh",
):
    np_bf = mybir.dt.np(mybir.dt.bfloat16)
    np_f8 = mybir.dt.np(mybir.dt.float8e4)
    hh_fp8 = fp8 == "zrhh"
    f = np.asarray(decoder_feature, np.float32)
    h0 = np.asarray(init_state, np.float32)
    o0 = np.asarray(decoder_init_input, np.float32)
    kx = np.asarray(kernel, np.float32)
    wr = np.asarray(recurrent_kernel, np.float32)
    ib = np.asarray(input_bias, np.float32)
    rb = np.asarray(recurrent_bias, np.float32)
    dw = np.asarray(dense_w, np.float32)
    db = float(np.asarray(dense_b, np.float32).reshape(-1)[0])
    k0 = kx[0]
    if hh_fp8:
        assert np.abs(rb[2 * H:]).max() == 0.0, \
            "fp8='zrhh' requires zero recurrent bias on the h gate"

    # bias row of the x-side stationary matrix: ib+rb for the z/r gate
    # columns (their mh/mx sum), ib only for the h columns (hh is biased
    # separately with rb inside the r* term), plus db routed through the
    # o-row weight (x row OROW carries o_raw = o - db).
    bias_row = np.concatenate([(ib + rb)[: 2 * H] * SC, ib[2 * H:]])
    bias_row += db * np.concatenate([k0[: 2 * H] * SC, k0[2 * H:]])
    kxw = np.zeros((XROWS, G3), np.float32)
    kxw[0:F, : 2 * H] = kx[1:, : 2 * H] * SC
    kxw[0:F, 2 * H:] = kx[1:, 2 * H:]
    kxw[ONESROW] = bias_row
    kxw[OROW, 2 * H:] = k0[2 * H:]  # o row: h cols only (z/r folded)

    # z/r columns of the recurrent weights with the o-feedback fold
    wr_zr = wr[:, : 2 * H] + dw @ k0[None, : 2 * H]

    def chunked(w, scale, dtype):
        # [256, C] -> [128, 2, C] with chunk c = rows [c*128, (c+1)*128)
        return np.ascontiguousarray(
            (w * scale).reshape(2, 128, w.shape[1]).transpose(1, 0, 2)
        ).astype(dtype)

    in_maps = []
    for i in range(ncores):
        s = slice(i * bl, (i + 1) * bl)
        featT = np.zeros((t_steps, XROWS, bl), np_bf)
        featT[:, 0:F, :] = f[s, :t_steps].transpose(1, 2, 0).astype(np_bf)
        featT[0, OROW, :] = (o0[s, 0] - db).astype(np_bf)
        featT[:, ONESROW, :] = 1.0
        h0c = np.ascontiguousarray(
            h0[s].T.reshape(2, 128, bl).transpose(1, 0, 2))
        in_maps.append({
            "featT": featT,
            "h0T": h0c.astype(np_bf),
            "h0T8": (h0c * SH).astype(np_f8),
            "kxw": kxw.astype(np_bf),
            "wrzr": chunked(wr_zr, SW, np_f8),
            "wrzr0": chunked(wr[:, : 2 * H], SW, np_f8),
            "wrh": chunked(wr[:, 2 * H:], SW if hh_fp8 else 1.0,
                           np_f8 if hh_fp8 else np_bf),
            "kxo": np.ascontiguousarray(k0[None, : 2 * H] * SC).astype(np_bf),
            "dww": np.ascontiguousarray(dw.reshape(2, 128).T).astype(np_bf),
            "rbh": np.ascontiguousarray(
                rb[2 * H:].reshape(2, 128).T).astype(np.float32),
        })
    return in_maps, db


def run(inputs: dict, compute_dt="bfloat16", nt=2, trace=False, trace_kwargs=None,
        fp8="zrhh", h8_pool=True, o_single=False):
    t_steps = int(inputs.get("predict_seq_length", T))
    assert t_steps == T, f"kernel hardcodes T={T}, got {t_steps}"
    nc = _get_nc(T, BL, nt, compute_dt, fp8, h8_pool, o_single)
    in_maps, db = make_in_maps(
        inputs["decoder_feature"], inputs["init_state"],
        inputs["decoder_init_input"], inputs["kernel"],
        inputs["recurrent_kernel"], inputs["input_bias"],
        inputs["recurrent_bias"], inputs["dense_w"], inputs["dense_b"],
        fp8=fp8,
    )
    res = bass_utils.run_bass_kernel_spmd(
        nc, in_maps, core_ids=list(range(NCORES)), trace=trace,
        **(trace_kwargs or {}),
    )
    out = np.empty((B, T, 1), np.float32)
    for i in range(NCORES):
        out[i * BL:(i + 1) * BL, :, 0] = \
            res.results[i]["outT"].T.astype(np.float32) + db
    return out, res


def kernel(**inputs) -> np.ndarray:
    out, _ = run(inputs)
    return out


# revision 14
# speedup vs baseline: 2.8145x; 2.8145x over previous
"""Trainium2 Bass kernel for nn_Decoder2 (GRU decoder, Keras reset_after GRUCell).

Reference computation (per batch row b, scanned over t = 0..T-1):
    x_t   = [o_{t-1}, feat_t]                  # [1+F]
    mx    = x_t @ K + ib                       # [3H]
    mh    = h_{t-1} @ Wr + rb                  # [3H]
    z     = sigmoid(mx[:H]   + mh[:H])
    r     = sigmoid(mx[H:2H] + mh[H:2H])
    cand  = tanh(mx[2H:] + r * mh[2H:])
    h_t   = z * h_{t-1} + (1-z) * cand
    o_t   = h_t @ dw + db                      # scalar output per row

Shapes: B=8192, T=96, F=64, H=256.

Strategy: pure data parallel over batch (1024 rows/core on 8 cores), no
collectives.  On-chip layout is feature-major: [feature -> partitions,
batch -> free dim], so TensorE consumes the tiny weights as stationary
operands and the bias rides along as a ones-row in x.

Perf structure:
  * o-feedback for the z/r gates is folded host-side into the recurrent
    weights (Wr' = Wr + dw@k0^T on the z/r columns), shortening the
    per-step latency chain.  Only the xh pass reads the o row of x.
  * the recurrent gate matmuls run in fp8e4m3 with the DoubleRow perf
    mode: one K=256 pass per 128-col chunk at 0.5 cycles/row (4x fewer
    PE cycles than bf16).  Scales: h*16, wr*32 -> PSUM holds 512x the
    true preactivation; the x-side z/r weights are pre-scaled by 512 to
    match and the sigmoid undoes it via the ACT scale argument.  The
    output path (o) stays bf16.
  * x tile is [66, n]: feat rows 0..63, o row 64, ones row 65.  The
    ones row ships from the host inside featT (no per-step memsets).
  * no separate "xh += r*hh" DVE add: DVE writes rh = r*(hh+rb)
    directly into the xh PSUM bank, and the xh matmul (deferred to the
    next pipeline block so the PE never waits on DVE) accumulates the
    x-side on top with start=False.
  * z and r PSUM share one 4-bank tile; sigmoid(r) is issued first
    (it is on the recurrence critical path), sigmoid(z) after.
  * elementwise work spread across DVE (rh, blends), Pool (fp8 cast of
    h, o->x-row cast), ACT (sigmoids/tanh/o copy).
"""

import os
import sys

for _p in ("/root/.axon_site/_ro/trn_rl_repo", "/opt/trn_rl_repo"):
    if os.path.isdir(_p) and _p not in sys.path:
        sys.path.insert(0, _p)

from contextlib import ExitStack  # noqa: E402

import numpy as np  # noqa: E402

import concourse.bacc as bacc  # noqa: E402
import concourse.tile as tile  # noqa: E402
from concourse import mybir  # noqa: E402
from concourse import bass_utils  # noqa: E402

Alu = mybir.AluOpType
Act = mybir.ActivationFunctionType
DR = mybir.MatmulPerfMode.DoubleRow

B, T, F, H = 8192, 96, 64, 256
G3 = 3 * H              # 768 gate width
NCORES = 8
BL = B // NCORES        # 1024 batch rows per core
# x tile partition layout:
#   rows 0..63  = feat_t^T
#   row  64     = o_{t-1} (prev dense output, raw = o - db)
#   row  65     = ones (bias row, shipped from host inside featT)
XROWS = 66
OROW = 64
ONESROW = 65
SH = 1.0                # fp8 scale on h (1.0: the fp8 copy of h is a plain
                        # cast, produced as a second a-b subtract on DVE)
SW = 32.0               # fp8 scale on the fp8 recurrent weights
SC = SH * SW            # combined PSUM scale on the z/r preactivations


def build_nc(
    t_steps: int = T,
    bl: int = BL,
    nt: int = 2,
    fp8: str = "zrhh",        # "zr" | "zrhh"  (recurrent matmul precision)
    h8_pool: bool = True,     # fp8 cast of h on gpsimd (else DVE)
    o_single: bool = False,   # single bf16 o copy (outT in bf16)
):
    """Build (and compile) the per-core Bass program.

    With fp8="zrhh" the recurrent bias rb must be zero (asserted in
    make_in_maps): the hh DoubleRow matmul has nowhere to add it.
    """
    n = bl // nt                     # batch-tile free size (<=512: one PSUM bank)
    assert n <= 512
    bf = mybir.dt.bfloat16
    f8 = mybir.dt.float8e4
    f32 = mybir.dt.float32
    hh_fp8 = fp8 == "zrhh"
    nch = H // 128                   # 2 chunks of 128 features for h
    assert nch == 2

    nc = bacc.Bacc("TRN2", target_bir_lowering=False, debug=False)

    featT = nc.dram_tensor("featT", [t_steps, XROWS, bl], bf, kind="ExternalInput").ap()
    h0T = nc.dram_tensor("h0T", [128, nch, bl], bf, kind="ExternalInput").ap()
    h0T8 = nc.dram_tensor("h0T8", [128, nch, bl], f8, kind="ExternalInput").ap()
    # kxw: [66, G3] x-side stationary.  z/r columns pre-scaled by SC.
    # row OROW: zeros for z/r columns (o-feedback folded into wrzr there),
    # kernel[0] for the h columns.  row ONESROW carries the biases.
    kxw = nc.dram_tensor("kxw", [XROWS, G3], bf, kind="ExternalInput").ap()
    # z/r recurrent weights (folded o-feedback), fp8 *SW
    wrzr = nc.dram_tensor("wrzr", [128, nch, 2 * H], f8, kind="ExternalInput").ap()
    # unfolded variant for t=0 (o-feedback comes via kxo from the x row)
    wrzr0 = nc.dram_tensor("wrzr0", [128, nch, 2 * H], f8, kind="ExternalInput").ap()
    # h-column recurrent weights
    wrh = nc.dram_tensor("wrh", [128, nch, H], f8 if hh_fp8 else bf,
                         kind="ExternalInput").ap()
    # k0 z/r columns (*SC) for the t=0 K=1 o-feedback matmul
    kxo = nc.dram_tensor("kxo", [1, 2 * H], bf, kind="ExternalInput").ap()
    dww = nc.dram_tensor("dww", [128, nch], bf, kind="ExternalInput").ap()
    ident = nc.dram_tensor("ident", [128, 128], bf, kind="ExternalInput").ap()
    rbh = nc.dram_tensor("rbh", [128, nch], f32, kind="ExternalInput").ap()
    out_dt = bf if o_single else f32
    outT = nc.dram_tensor("outT", [t_steps, bl], out_dt, kind="ExternalOutput").ap()

    with tile.TileContext(nc) as tc, ExitStack() as ctx:
        const = ctx.enter_context(tc.tile_pool(name="const", bufs=1))
        hpool = ctx.enter_context(tc.tile_pool(name="h", bufs=3))
        h8pool = ctx.enter_context(tc.tile_pool(name="h8", bufs=3))
        xpool = ctx.enter_context(tc.tile_pool(name="x", bufs=6))
        zrpool = ctx.enter_context(tc.tile_pool(name="zr", bufs=3))
        rhpool = ctx.enter_context(tc.tile_pool(name="rh", bufs=3))
        apool = ctx.enter_context(tc.tile_pool(name="a", bufs=3))
        bpool = ctx.enter_context(tc.tile_pool(name="b", bufs=3))
        cpool = ctx.enter_context(tc.tile_pool(name="cand", bufs=3))
        opool = ctx.enter_context(tc.tile_pool(name="osb", bufs=4))
        # PSUM: 4 (zr) + 2 (hh, shared with o) + 2 (xq) = 8 banks
        pzr = ctx.enter_context(tc.tile_pool(name="pzr", bufs=1, space="PSUM"))
        phh = ctx.enter_context(tc.tile_pool(name="phh", bufs=1, space="PSUM"))
        pxq = ctx.enter_context(tc.tile_pool(name="pxq", bufs=1, space="PSUM"))

        # --- constants ---
        kx_sb = const.tile([XROWS, G3], bf)
        nc.sync.dma_start(out=kx_sb, in_=kxw)
        wrzr_sb = const.tile([128, nch, 2 * H], f8)
        nc.sync.dma_start(out=wrzr_sb, in_=wrzr)
        wrzr0_sb = const.tile([128, nch, 2 * H], f8)
        nc.sync.dma_start(out=wrzr0_sb, in_=wrzr0)
        wrh_sb = const.tile([128, nch, H], f8 if hh_fp8 else bf)
        nc.sync.dma_start(out=wrh_sb, in_=wrh)
        # k0 z/r row staged at partition OROW so the t=0 K=1 matmul's
        # stationary and moving operands share a row group
        kxo_sb = const.tile([XROWS, 2 * H], bf)
        nc.sync.dma_start(out=kxo_sb[OROW:OROW + 1, :], in_=kxo)
        dw_sb = const.tile([128, nch], bf)
        nc.sync.dma_start(out=dw_sb, in_=dww)
        id_sb = const.tile([128, 128], bf)
        nc.sync.dma_start(out=id_sb, in_=ident)
        rb_sb = const.tile([128, nch], f32)
        nc.sync.dma_start(out=rb_sb, in_=rbh)

        # --- initial state ---
        h_prev0 = hpool.tile([128, nch, bl], bf)
        nc.sync.dma_start(out=h_prev0, in_=h0T)
        h8_prev0 = h8pool.tile([128, nch, bl], f8)
        nc.sync.dma_start(out=h8_prev0, in_=h0T8)
        xs = {}
        for j in range(nt):
            xj = xpool.tile([XROWS, n], bf, tag="x")
            nc.sync.dma_start(out=xj, in_=featT[0, :, j * n:(j + 1) * n])
            xs[(0, j)] = xj

        h_hist = {-1: h_prev0}
        h8_hist = {-1: h8_prev0}
        zr_hist = {}
        xq_hist = {}
        rh_hist = {}

        def emit_tail_mm(t, j):
            """PE half of the deferred tail of block (t, j): the xh matmul
            opens the psum group and identity matmuls accumulate rh on top.
            Deferred one block so the PE never waits on DVE."""
            bs = slice(j * n, (j + 1) * n)
            x = xs[(t, j)]
            rh_sb = rh_hist.pop((t, j))
            xq = pxq.tile([128, nch, n], f32, tag="pxq")
            xq_hist[(t, j)] = xq
            for c in range(nch):
                m = 2 * H + c * 128
                nc.tensor.matmul(xq[:, c, :], kx_sb[:, m:m + 128], x,
                                 start=True, stop=False)
                nc.tensor.matmul(xq[:, c, :], id_sb, rh_sb[:, c, :],
                                 start=False, stop=True)

        def emit_tail(t, j, skip_h8=False):
            """ACT/DVE half of the deferred tail: tanh, blend, fp8 cast."""
            bs = slice(j * n, (j + 1) * n)
            xq = xq_hist.pop((t, j))
            zr_sb = zr_hist.pop((t, j))
            z_sb = zr_sb[:, 0:nch, :]
            h_prev = h_hist[t - 1]
            h_new = h_hist[t]
            h8_new = h8_hist[t]
            cand = cpool.tile([128, nch, n], bf, tag="cand")
            nc.scalar.activation(cand, xq, Act.Tanh)
            # h_new = A - (z-1)*cand  with A = z*h (off the tanh chain)
            a_sb = apool.tile([128, nch, n], bf, tag="a")
            nc.vector.tensor_tensor(a_sb, z_sb, h_prev[:, :, bs], Alu.mult)
            b_sb = bpool.tile([128, nch, n], bf, tag="b")
            nc.vector.scalar_tensor_tensor(
                b_sb, z_sb, -1.0, cand, Alu.add, Alu.mult)
            # fp8 state copy first: it heads the next step's critical path
            if not skip_h8:
                nc.vector.tensor_tensor(h8_new[:, :, bs], a_sb, b_sb,
                                        Alu.subtract)
            nc.vector.tensor_tensor(h_new[:, :, bs], a_sb, b_sb, Alu.subtract)

        def emit_o_mm(j, h_t):
            """o(t,j) = h(t,j) @ dw into a psum slot shared with hh."""
            bs = slice(j * n, (j + 1) * n)
            op = phh.tile([1, n], f32, tag="phh")
            nc.tensor.matmul(op[0:1, :], dw_sb[:, 0:1], h_t[:, 0, bs],
                             start=True, stop=False)
            nc.tensor.matmul(op[0:1, :], dw_sb[:, 1:2], h_t[:, 1, bs],
                             start=False, stop=True)
            return op

        def emit_o_out(t, j, op, x_next):
            """Copy o(t,j) out of PSUM: to outT and (if x_next) into the o
            row of x(t+1,j).  The PSUM->SBUF copy alternates ACT/DVE per j
            to balance engine load."""
            bs = slice(j * n, (j + 1) * n)
            if o_single:
                if x_next is not None:
                    dst = x_next[OROW:OROW + 1, :]
                else:
                    dst = opool.tile([1, n], bf, tag="osb", name="osb_tail")
                if j == 0:
                    nc.scalar.activation(dst, op, Act.Copy)
                else:
                    nc.vector.tensor_copy(out=dst, in_=op)
                nc.sync.dma_start(out=outT[t:t + 1, bs], in_=dst)
            else:
                o_sb = opool.tile([1, n], f32, tag="osb")
                if j == 0:
                    nc.scalar.activation(o_sb, op, Act.Copy)
                else:
                    nc.vector.tensor_copy(out=o_sb, in_=op)
                nc.sync.dma_start(out=outT[t:t + 1, bs], in_=o_sb)
                if x_next is not None:
                    nc.vector.tensor_copy(out=x_next[OROW:OROW + 1, :],
                                          in_=o_sb)

        prev = None
        for t in range(t_steps):
            h_new = hpool.tile([128, nch, bl], bf, tag="h")
            h8_new = h8pool.tile([128, nch, bl], f8, tag="h8")
            h_hist[t] = h_new
            h8_hist[t] = h8_new
            h_prev = h_hist[t - 1]
            h8_prev = h8_hist[t - 1]
            for j in range(nt):
                bs = slice(j * n, (j + 1) * n)
                x = xs[(t, j)]

                # 1. z/r gate matmuls: fp8 DoubleRow recurrent pass + bf16
                # x pass, in a shared 4-bank psum tile (z chunks 0..1,
                # r chunks 2..3).
                zrp = pzr.tile([128, 2 * nch, n], f32, tag="pzr")
                wsrc = wrzr0_sb if t == 0 else wrzr_sb
                for g in range(2 * nch):        # z0 z1 r0 r1
                    m = g * 128
                    nc.tensor.matmul(zrp[:, g, :], wsrc[:, :, m:m + 128],
                                     h8_prev[:, :, bs],
                                     start=True, stop=False, perf_mode=DR)
                    nc.tensor.matmul(zrp[:, g, :], kx_sb[:, m:m + 128], x,
                                     start=False, stop=t != 0)
                    if t == 0:
                        # o-feedback at t=0 is the external init input
                        # (x row OROW); K=1 matmul on PE row-group 64
                        nc.tensor.matmul(
                            zrp[:, g, :], kxo_sb[OROW:OROW + 1, m:m + 128],
                            x[OROW:OROW + 1, :],
                            start=False, stop=True, tile_position=(64, 0))

                # 2. PE half of the previous block's tail (xh + rh add)
                if prev is not None:
                    emit_tail_mm(*prev)

                # 3. deferred output work from step t-1 (h ready long ago;
                # the o copy must precede the hh matmuls: shared psum slot)
                if t > 0:
                    op = emit_o_mm(j, h_prev)
                    emit_o_out(t - 1, j, op, x)

                # 4. hh matmuls
                hhp = phh.tile([128, nch, n], f32, tag="phh")
                for c in range(nch):
                    if hh_fp8:
                        nc.tensor.matmul(
                            hhp[:, c, :], wrh_sb[:, :, c * 128:c * 128 + 128],
                            h8_prev[:, :, bs], start=True, stop=True,
                            perf_mode=DR)
                    else:
                        nc.tensor.matmul(
                            hhp[:, c, :], wrh_sb[:, 0, c * 128:c * 128 + 128],
                            h_prev[:, 0, bs], start=True, stop=False)
                        nc.tensor.matmul(
                            hhp[:, c, :], wrh_sb[:, 1, c * 128:c * 128 + 128],
                            h_prev[:, 1, bs], start=False, stop=True)

                # 5. merged sigmoid over z and r (one 4-bank read)
                zr_sb = zrpool.tile([128, 2 * nch, n], bf, tag="zr")
                nc.scalar.activation(zr_sb, zrp, Act.Sigmoid, scale=1.0 / SC)
                r_sb = zr_sb[:, nch:2 * nch, :]

                # 6. ACT/DVE half of the previous block's tail
                if prev is not None:
                    emit_tail(*prev)

                # 7. rh = r * (hh + rb) -> SBUF; the next block's
                # identity matmuls fold it into the xh psum group.
                rh_sb = rhpool.tile([128, nch, n], bf, tag="rh")
                if hh_fp8:
                    nc.vector.scalar_tensor_tensor(
                        rh_sb, hhp, 1.0 / SC, r_sb, Alu.mult, Alu.mult)
                else:
                    for c in range(nch):
                        nc.vector.scalar_tensor_tensor(
                            rh_sb[:, c, :], hhp[:, c, :], rb_sb[:, c:c + 1],
                            r_sb[:, c, :], Alu.add, Alu.mult)
                rh_hist[(t, j)] = rh_sb
                zr_hist[(t, j)] = zr_sb
                prev = (t, j)

                # 8. prefetch next step's x tile (feat + o slot + ones)
                if t < t_steps - 1:
                    xj = xpool.tile([XROWS, n], bf, tag="x")
                    nc.sync.dma_start(
                        out=xj, in_=featT[t + 1, :, j * n:(j + 1) * n])
                    xs[(t + 1, j)] = xj

            h_hist.pop(t - 2, None)
            h8_hist.pop(t - 2, None)
            xs.pop((t - 1, 0), None)
            xs.pop((t - 1, 1), None)

        # flush: tail of the last block, then the last step's outputs
        emit_tail_mm(t_steps - 1, nt - 1)
        emit_tail(t_steps - 1, nt - 1, skip_h8=True)
        for j in range(nt):
            op = emit_o_mm(j, h_hist[t_steps - 1])
            emit_o_out(t_steps - 1, j, op, None)

    nc.compile()
    return nc


_NC_CACHE: dict = {}


def _get_nc(t_steps=T, bl=BL, nt=2, compute_dt="bfloat16", fp8="zrhh",
            h8_pool=True, o_single=False):
    key = (t_steps, bl, nt, fp8, h8_pool, o_single)
    if key not in _NC_CACHE:
        _NC_CACHE[key] = build_nc(t_steps, bl, nt, fp8, h8_pool, o_single)
    return _NC_CACHE[key]


def make_in_maps(
    decoder_feature,
    init_state,
    decoder_init_input,
    kernel,
    recurrent_kernel,
    input_bias,
    recurrent_bias,
    dense_w,
    dense_b,
    t_steps=T,
    bl=BL,
    ncores=NCORES,
    fp8="zrhh",
):
    np_bf = mybir.dt.np(mybir.dt.bfloat16)
    np_f8 = mybir.dt.np(mybir.dt.float8e4)
    hh_fp8 = fp8 == "zrhh"
    f = np.asarray(decoder_feature, np.float32)
    h0 = np.asarray(init_state, np.float32)
    o0 = np.asarray(decoder_init_input, np.float32)
    kx = np.asarray(kernel, np.float32)
    wr = np.asarray(recurrent_kernel, np.float32)
    ib = np.asarray(input_bias, np.float32)
    rb = np.asarray(recurrent_bias, np.float32)
    dw = np.asarray(dense_w, np.float32)
    db = float(np.asarray(dense_b, np.float32).reshape(-1)[0])
    k0 = kx[0]
    if hh_fp8:
        assert np.abs(rb[2 * H:]).max() == 0.0, \
            "fp8='zrhh' requires zero recurrent bias on the h gate"

    # bias row of the x-side stationary matrix: ib+rb for the z/r gate
    # columns (their mh/mx sum), ib only for the h columns (hh is biased
    # separately with rb inside the r* term), plus db routed through the
    # o-row weight (x row OROW carries o_raw = o - db).
    bias_row = np.concatenate([(ib + rb)[: 2 * H] * SC, ib[2 * H:]])
    bias_row += db * np.concatenate([k0[: 2 * H] * SC, k0[2 * H:]])
    kxw = np.zeros((XROWS, G3), np.float32)
    kxw[0:F, : 2 * H] = kx[1:, : 2 * H] * SC
    kxw[0:F, 2 * H:] = kx[1:, 2 * H:]
    kxw[ONESROW] = bias_row
    kxw[OROW, 2 * H:] = k0[2 * H:]  # o row: h cols only (z/r folded)

    # z/r columns of the recurrent weights with the o-feedback fold
    wr_zr = wr[:, : 2 * H] + dw @ k0[None, : 2 * H]

    def chunked(w, scale, dtype):
        # [256, C] -> [128, 2, C] with chunk c = rows [c*128, (c+1)*128)
        return np.ascontiguousarray(
            (w * scale).reshape(2, 128, w.shape[1]).transpose(1, 0, 2)
        ).astype(dtype)

    in_maps = []
    for i in range(ncores):
        s = slice(i * bl, (i + 1) * bl)
        featT = np.zeros((t_steps, XROWS, bl), np_bf)
        featT[:, 0:F, :] = f[s, :t_steps].transpose(1, 2, 0).astype(np_bf)
        featT[0, OROW, :] = (o0[s, 0] - db).astype(np_bf)
        featT[:, ONESROW, :] = 1.0
        h0c = np.ascontiguousarray(
            h0[s].T.reshape(2, 128, bl).transpose(1, 0, 2))
        in_maps.append({
            "featT": featT,
            "h0T": h0c.astype(np_bf),
            "h0T8": (h0c * SH).astype(np_f8),
            "kxw": kxw.astype(np_bf),
            "wrzr": chunked(wr_zr, SW, np_f8),
            "wrzr0": chunked(wr[:, : 2 * H], SW, np_f8),
            "wrh": chunked(wr[:, 2 * H:], SW if hh_fp8 else 1.0,
                           np_f8 if hh_fp8 else np_bf),
            "kxo": np.ascontiguousarray(k0[None, : 2 * H] * SC).astype(np_bf),
            "dww": np.ascontiguousarray(dw.reshape(2, 128).T).astype(np_bf),
            "ident": np.eye(128, dtype=np.float32).astype(np_bf),
            "rbh": np.ascontiguousarray(
                rb[2 * H:].reshape(2, 128).T).astype(np.float32),
        })
    return in_maps, db


def run(inputs: dict, compute_dt="bfloat16", nt=2, trace=False, trace_kwargs=None,
        fp8="zrhh", h8_pool=True, o_single=False):
    t_steps = int(inputs.get("predict_seq_length", T))
    assert t_steps == T, f"kernel hardcodes T={T}, got {t_steps}"
    nc = _get_nc(T, BL, nt, compute_dt, fp8, h8_pool, o_single)
    in_maps, db = make_in_maps(
        inputs["decoder_feature"], inputs["init_state"],
        inputs["decoder_init_input"], inputs["kernel"],
        inputs["recurrent_kernel"], inputs["input_bias"],
        inputs["recurrent_bias"], inputs["dense_w"], inputs["dense_b"],
        fp8=fp8,
    )
    res = bass_utils.run_bass_kernel_spmd(
        nc, in_maps, core_ids=list(range(NCORES)), trace=trace,
        **(trace_kwargs or {}),
    )
    out = np.empty((B, T, 1), np.float32)
    for i in range(NCORES):
        out[i * BL:(i + 1) * BL, :, 0] = \
            res.results[i]["outT"].T.astype(np.float32) + db
    return out, res


def kernel(**inputs) -> np.ndarray:
    out, _ = run(inputs)
    return out
